# revision 2
# baseline (speedup 1.0000x reference)
"""Trainium2 Bass kernel for nn_Fast2Order_DE_Conv.

Math: out[b,o,ho,wo] = sum_{c,i,j} W[o, c*81+i*9+j] * p_i * p_j with
p_i = x[b, c, ho+di, wo+dj] (i = di*3+dj, 3x3 unfold of a 16-channel 64x64
image; output 62x62).

Algorithm: change the quadratic-feature basis from products p_i*p_j to
squares {p_i^2, (p_i+p_j)^2, i<j} (45 per channel, 720 total) and fold the
basis change into W on the host (W2 = W * M^-1).  On-chip, per spatial tile
of 512 locations:

    selection matmul (PE, f16):  s = AselT.T @ x_unfold  [768 padded rows]
    square          (ACT/DVE):   g = s^2, PSUM -> SBUF f16
    main matmul     (PE, f16):   out += W2T.T @ g, accumulated in fp32 PSUM

All matmuls use float16 (e5m10: ~f32r accuracy at half the width, 2-byte
FWL-eligible weight loads, full PE rate).  Inputs are cast to f16 on the
host so DMA loads feed the PE directly.  The 3x3 unfold itself is free: it
is expressed in the DMA access pattern (overlapping windows of the padded
l' = ho*64+wo layout).

Pipelining: tiles are software-pipelined with skew 3 (a tile's selection
matmuls + squares issue three tiles before its main matmuls) so the PE
never waits on the square engines; a burst of warmup matmuls during the
initial DMA window keeps the PE clock gate at full rate.  Per-core device
time ~55 us, ~80% of the PE streaming roofline; the residue is per-matmul
weight-load and dispatch overhead.

Sharding: data-parallel over batch, 2 batches per core on 8 cores; W-side
constants are replicated.  Output gathered by simple concatenation.
"""

import functools

import numpy as np

import concourse.bacc as bacc
import concourse.mybir as mybir
from concourse.tile import TileContext
from concourse.bass_utils import run_bass_kernel_spmd

B, C, H, WIDTH = 16, 16, 64, 64
O = 128
HO = WO = 62
N_CORES = 8
B_LOC = B // N_CORES
PAIRS = [(i, j) for i in range(9) for j in range(i, 9)]  # 45
ROW_TILES = [(0, 8), (8, 8), (16, 8), (24, 8), (32, 8), (40, 8), (48, 8), (56, 6)]
NCHUNK = 6  # g chunks of 128 rows (768 total, 48 zero-padded)
GC = 128
GH = 384  # padded g rows per c-half (360 real + 24 pad)


def _round_f32r(a: np.ndarray) -> np.ndarray:
    """Round fp32 values to the f32r grid (RNE at 12 low mantissa bits)."""
    a = np.ascontiguousarray(a, dtype=np.float32)
    bits = a.view(np.uint32).astype(np.uint64)
    half, mask = np.uint64(0x800), np.uint64(0xFFF)
    lsb = (bits >> np.uint64(12)) & np.uint64(1)
    out = ((bits + half - np.uint64(1) + lsb) & ~mask).astype(np.uint32)
    return out.view(np.float32).reshape(a.shape)


def _build_consts(Wf: np.ndarray):
    """W (128, 1296) -> (AselT [72, 360] f32, W2T [720, 128] f32, f32r grid)."""
    Wt = np.asarray(Wf, dtype=np.float64).reshape(O, C, 9, 9)
    Wsym = Wt + Wt.transpose(0, 1, 3, 2)
    W2 = np.zeros((O, 720))
    for c in range(C):
        for pi, (i, j) in enumerate(PAIRS):
            f = c * 45 + pi
            if i == j:
                W2[:, f] = Wt[:, c, i, i] - 0.5 * (
                    Wsym[:, c, i, :].sum(-1) - 2.0 * Wt[:, c, i, i]
                )
            else:
                W2[:, f] = 0.5 * Wsym[:, c, i, j]
    # x-row layout on chip: row = i*8 + c_local (i = di*3+dj kernel position)
    AselT = np.zeros((72, 384), dtype=np.float32)
    for cl in range(8):
        for pi, (i, j) in enumerate(PAIRS):
            g = cl * 45 + pi
            AselT[i * 8 + cl, g] += 1.0
            if i != j:
                AselT[j * 8 + cl, g] += 1.0
    # pad each c-half's 360 features to 384 (3 chunks of 128) so every
    # selection matmul has exactly 128 stationary columns (enables FWL)
    W2p = np.zeros((O, 768))
    W2p[:, 0:360] = W2[:, 0:360]
    W2p[:, 384:744] = W2[:, 360:720]
    W2T = np.ascontiguousarray(W2p.T).astype(np.float16)  # [768, 128]
    return AselT.astype(np.float16), W2T


def _x_window_ap(x_d, b: int, h: int, ho0: int, di: int, lt_load: int):
    """Source AP for one di of the unfold load: (dj, c, l) nesting matching
    target partitions (di*3+dj)*8 + c, free dim = padded l' = ho*64+wo."""
    ap = x_d[b, h * 8 : (h + 1) * 8, ho0 + di, 0:3].unsqueeze(-1)
    v = ap.ap
    v[0] = [1, 3]
    v[1] = [H * WIDTH, 8]
    v[2] = [1, lt_load]
    return ap


def build_nc(reps: int = 1, skew: int = 3):
    """Build the per-core program.  reps>1 wraps the body in an on-chip loop
    (used only for device-time measurement); skew is the software-pipeline
    depth between a tile's selection/squares and its main matmuls."""
    f32, f16 = mybir.dt.float32, mybir.dt.float16
    nc = bacc.Bacc("TRN2", target_bir_lowering=False)
    x_d = nc.dram_tensor("x_loc", [B_LOC, C, H, WIDTH], f16, kind="ExternalInput")
    a_d = nc.dram_tensor("aselT", [72, GH], f16, kind="ExternalInput")
    w_d = nc.dram_tensor("w2T", [2 * GH, O], f16, kind="ExternalInput")
    o_d = nc.dram_tensor("out_loc", [B_LOC, O, HO, WO], f32, kind="ExternalOutput")

    with TileContext(nc) as tc:
        with (
            tc.tile_pool(name="const", bufs=1) as cpool,
            tc.tile_pool(name="xin", bufs=2) as xpool,
            tc.tile_pool(name="gbuf", bufs=3 * (skew + 1) + 3) as gpool,
            tc.tile_pool(name="tmpbuf", bufs=4) as tmppool,
            tc.tile_pool(name="obuf", bufs=6) as opool,
            tc.tile_pool(name="ps_sel", bufs=3, space="PSUM") as pspool,
            tc.tile_pool(name="ps_out", bufs=2, space="PSUM") as popool,
        ):
            LFULL = HO * 64  # 3968 columns of the padded l' = ho*64+wo axis

            a_r = cpool.tile([72, GH], f16, tag="a_r")
            nc.sync.dma_start(a_r[:], a_d[:])

            def load_x(x_t, b, h, col0, col1, eng=None):
                """Fill x_t[:, col0:col1] of the unfold view for (b, c-half h)."""
                eng = eng or nc.sync
                for di in range(3):
                    hi = min(col1, H * WIDTH - di * 64 - 2)
                    if hi > col0:
                        ap = _x_window_ap(x_d, b, h, 0, di, hi - col0)
                        ap.offset += col0
                        eng.dma_start(x_t[di * 24 : (di + 1) * 24, col0:hi], ap)
                    if hi < col1:
                        # pad columns feed discarded outputs; fill with
                        # arbitrary valid f32r data to keep reads clean
                        eng.dma_start(
                            x_t[di * 24 : (di + 1) * 24, hi:col1],
                            _x_window_ap(x_d, b, h, 0, 0, col1 - hi),
                        )

            # all unfold loads up front; batch 0 split so tile 0 starts early
            xr_all = []
            for b in range(B_LOC):
                xr_b = []
                for h in range(2):
                    x_t = xpool.tile([72, LFULL], f16, tag=f"x{h}", name=f"x{h}_{b}")
                    xr_b.append(x_t)
                xr_all.append(xr_b)
            for h in range(2):
                load_x(xr_all[0][h], 0, h, 0, 1024)
            w_r = cpool.tile([GC, NCHUNK, O], f16, tag="w_r")
            nc.sync.dma_start(w_r[:], w_d[:].rearrange("(k p) o -> p k o", p=GC))
            for h in range(2):
                load_x(xr_all[0][h], 0, h, 1024, LFULL)
            for b in range(1, B_LOC):
                for h in range(2):
                    load_x(xr_all[b][h], b, h, 0, LFULL)

            # greedy ACT/DVE load balancing for PSUM-draining elementwise
            # ops (DVE pays double for squares: bounce + SBUF square)
            eng_busy = {"act": 0.0, "dve": 0.0}

            def square_merged(g_t, ps_s, lt):
                gv = g_t[:, :, :lt]
                pv = ps_s[:, :, :lt]
                if eng_busy["act"] + 1.0 <= eng_busy["dve"] + 2.1:
                    nc.scalar.square(gv, pv)
                    eng_busy["act"] += 1.0
                else:
                    tmp = tmppool.tile([GC, 2, 512], f32, tag="sq_tmp")
                    tv = tmp[:, :, :lt]
                    nc.vector.tensor_copy(tv, pv)
                    nc.vector.tensor_mul(gv, tv, tv)
                    eng_busy["dve"] += 2.1

            def out_copy(o_view, ps_view):
                if eng_busy["act"] + 0.9 < eng_busy["dve"] + 0.55:
                    nc.scalar.copy(o_view, ps_view)
                    eng_busy["act"] += 0.9
                else:
                    nc.vector.tensor_copy(o_view, ps_view)
                    eng_busy["dve"] += 0.55

            def do_mains(st):
                """Main matmuls + drain for a tile whose squares are issued."""
                b, ho0, nr, g_ts = st
                lt = nr * 64
                ps_o = popool.tile([O, 512], f32, tag="ps_o", name="ps_o")
                for kk in range(NCHUNK):
                    nc.tensor.matmul(
                        ps_o[:, :lt],
                        w_r[:, kk, :],
                        g_ts[kk // 2][:, kk % 2, :lt],
                        start=(kk == 0),
                        stop=(kk == NCHUNK - 1),
                    )
                # compact to [O, nr*62] so the store uses contiguous chunks
                o_t = opool.tile([O, 8 * WO], f32, tag="o", name="o_t")
                ps_view = ps_o[:, :lt].rearrange("o (r w) -> o r w", w=64)
                o_view = o_t[:, : nr * WO].rearrange("o (r w) -> o r w", w=WO)
                out_copy(o_view, ps_view[:, :, :WO])
                nc.gpsimd.dma_start(
                    o_d[b, :, ho0 : ho0 + nr, :],
                    o_t[:, : nr * WO],
                )

            # HAM warmup: keep the PE busy during the initial DMA wait so the
            # clock gate is at 8/8 when real matmuls start (dummy MMs on the
            # first tile that lands; outputs never read)
            def warmup():
                for i in range(12):
                    ps_w = popool.tile([O, 512], f32, tag="ps_o", name="warm")
                    nc.tensor.matmul(
                        ps_w[:, :360], a_r[:, :128], a_r[:, :360],
                        start=True, stop=True,
                    )

            def body(it=None, unroll=1):
                # software-pipeline skew: issue tile t's selections and
                # squares, then tile (t-skew)'s mains — squares get `skew`
                # tiles of slack before the PE needs their output
                pending = []
                for b in range(B_LOC):
                    xr = xr_all[b]
                    for ho0, nr in ROW_TILES:
                        lt = nr * 64
                        c0 = ho0 * 64
                        g_ts = []
                        for kp in range(NCHUNK // 2):
                            # two 120-row chunks share one 2-bank PSUM tile so
                            # one elementwise op drains both
                            ps_s = pspool.tile(
                                [GC, 2, 512], f32, tag="ps_s", name="ps_s"
                            )
                            for half in range(2):
                                kk = kp * 2 + half
                                h, k = divmod(kk, 3)
                                nc.tensor.matmul(
                                    ps_s[:, half, :lt],
                                    a_r[:, k * GC : (k + 1) * GC],
                                    xr[h][:, c0 : c0 + lt],
                                    start=True,
                                    stop=True,
                                )
                            g_t = gpool.tile(
                                [GC, 2, 512], f16, tag="g", name="g_t"
                            )
                            square_merged(g_t, ps_s, lt)
                            g_ts.append(g_t)
                        pending.append((b, ho0, nr, g_ts))
                        if len(pending) > skew:
                            do_mains(pending.pop(0))
                for st in pending:
                    do_mains(st)

            warmup()
            if reps == 1:
                body()
            else:
                hint = (
                    mybir.EngineType.PE,
                    mybir.EngineType.Activation,
                    mybir.EngineType.DVE,
                    mybir.EngineType.SP,
                    mybir.EngineType.Pool,
                )
                with tc.For_i(0, reps, 1, hint_engines=hint) as _it:
                    body()
    nc.compile()
    return nc


@functools.lru_cache(maxsize=1)
def _cached_nc():
    return build_nc()


def make_in_maps(x: np.ndarray, W: np.ndarray) -> list:
    x = np.asarray(x, dtype=np.float32)
    W = np.asarray(W, dtype=np.float32)
    AselT, W2T = _build_consts(W)
    x_r = x.astype(np.float16)
    return [
        {
            "x_loc": np.ascontiguousarray(x_r[k * B_LOC : (k + 1) * B_LOC]),
            "aselT": AselT,
            "w2T": W2T,
        }
        for k in range(N_CORES)
    ]


def kernel(x: np.ndarray, W: np.ndarray, _trace: bool = False):
    nc = _cached_nc()
    in_maps = make_in_maps(x, W)
    try:
        r = run_bass_kernel_spmd(
            nc, in_maps, core_ids=list(range(N_CORES)), trace=_trace
        )
    except Exception:
        # transient NRT_EXEC_UNIT_UNRECOVERABLE has been observed once on
        # this fabric; a fresh attempt recovers
        r = run_bass_kernel_spmd(
            nc, in_maps, core_ids=list(range(N_CORES)), trace=_trace
        )
    out = np.concatenate([m["out_loc"] for m in r.results], axis=0)
    if _trace:
        kernel.last_result = r
    return out


if __name__ == "__main__":
    rng = np.random.default_rng(0)
    x = rng.standard_normal((B, C, H, WIDTH), dtype=np.float32)
    W = rng.standard_normal((O, C * 81), dtype=np.float32)
    out = kernel(x, W)
    print("out shape", out.shape, out.dtype)



# revision 23
# speedup vs baseline: 2.4321x; 2.4321x over previous
"""Trainium2 Bass kernel for nn_Fast2Order_DE_Conv (hybrid basis v4).

Math: out[b,o,ho,wo] = sum_{c,i,j} W[o, c*81+i*9+j] * p_i * p_j with
p_i = x[b, c, ho+di, wo+dj] (i = di*3+dj, 3x3 unfold of a 16-channel 64x64
image; output 62x62).

Quadratic features are evaluated in a HYBRID basis that splits work across
every engine (per channel, pairs keyed by index distance d = j - i):
  d in {1,2,3}: direct products x_i * x_j          -> DVE tensor_tensor
  d = 0, i<8  : squares x_i^2                      -> GPSIMD tensor_tensor
  d in {4..8} + x_8^2: (x_i+x_j)^2 squares basis   -> PE selection matmul
                                                      + ACT square
The basis change is folded into W on the host.  Per 512-column spatial
tile the PE runs only 2 selection + 6 main matmuls (vs 12 in a pure
squares-basis kernel); the d<=3 product features are computed at full
3968-column width, with the shifted second operands staged by SBUF->SBUF
DMA into partition-aligned pack tiles (DVE ops cannot read unaligned
partition bases).

Sharding: data-parallel over batch, 2 batches per core on 8 cores.
"""

import functools

import numpy as np

import concourse.bacc as bacc
import concourse.mybir as mybir
from concourse.tile import TileContext
from concourse.bass_utils import run_bass_kernel_spmd

B, C, H, WIDTH = 16, 16, 64, 64
O = 128
HO = WO = 62
N_CORES = 8
B_LOC = B // N_CORES
TILE_PAIRS = [((0, 8), (8, 8)), ((16, 8), (24, 8)),
              ((32, 8), (40, 8)), ((48, 8), (56, 6))]
GC = 128
SEL_PAIRS = [(i, i + d) for d in range(4, 9) for i in range(0, 9 - d)]  # 15
DEDUP_LDW = True
D0_GPS = False
ABLATE = {"hoist_packs"}
D0_SPLIT = False
X_BUFS = 1
GF_BUFS = 8
GS_BUFS = 6
O_BUFS = 2
PK_BUFS = 1


def _build_consts(Wf: np.ndarray):
    """W (128, 1296) -> (AselT [72, 128] f16, W2T [768, 128] f16).

    Per half (8 channels), 384 rows = 3 chunks of 128:
      chunk0: rows i*8+cl    (i 0..7) -> product (i, i+1)   [64]
              rows 64+i*8+cl (i 0..6) -> product (i, i+2)   [56]
      chunk1: rows i*8+cl    (i 0..7) -> x_i^2              [64]
              rows 64+i*8+cl (i 0..5) -> product (i, i+3)   [48]
      chunk2: sel (x_i+x_j)^2 for (i,j) in SEL_PAIRS [120], x_8^2 [8]
    """
    Wt = np.asarray(Wf, dtype=np.float64).reshape(O, C, 9, 9)
    Wsym = Wt + Wt.transpose(0, 1, 3, 2)
    W2 = np.zeros((O, 2, 3, 128))
    AselT = np.zeros((72, 128))
    for h in range(2):
        for cl in range(8):
            c = h * 8 + cl
            for i in range(8):
                W2[:, h, 0, i * 8 + cl] = Wsym[:, c, i, i + 1]
            for i in range(7):
                W2[:, h, 0, 64 + i * 8 + cl] = Wsym[:, c, i, i + 2]
            for i in range(8):
                W2[:, h, 1, i * 8 + cl] = Wt[:, c, i, i] - 0.5 * sum(
                    Wsym[:, c, a, b] for (a, b) in SEL_PAIRS if i in (a, b)
                )
            for i in range(6):
                W2[:, h, 1, 64 + i * 8 + cl] = Wsym[:, c, i, i + 3]
            for pi, (i, j) in enumerate(SEL_PAIRS):
                W2[:, h, 2, pi * 8 + cl] = 0.5 * Wsym[:, c, i, j]
            W2[:, h, 2, 120 + cl] = Wt[:, c, 8, 8] - 0.5 * sum(
                Wsym[:, c, a, b] for (a, b) in SEL_PAIRS if 8 in (a, b)
            )
    for cl in range(8):
        for pi, (i, j) in enumerate(SEL_PAIRS):
            AselT[i * 8 + cl, pi * 8 + cl] = 1.0
            AselT[j * 8 + cl, pi * 8 + cl] += 1.0
        AselT[64 + cl, 120 + cl] = 1.0
    W2T = np.ascontiguousarray(
        W2.transpose(1, 2, 3, 0).reshape(768, O)
    ).astype(np.float16)
    return AselT.astype(np.float16), W2T


def _x_window_ap(x_d, b: int, h: int, di: int, lt_load: int):
    """Source AP for one di of the unfold load: (dj, c, l) nesting matching
    target partitions (di*3+dj)*8 + c, free dim = padded l' = ho*64+wo."""
    ap = x_d[b, h * 8 : (h + 1) * 8, di, 0:3].unsqueeze(-1)
    v = ap.ap
    v[0] = [1, 3]
    v[1] = [H * WIDTH, 8]
    v[2] = [1, lt_load]
    return ap


def _dedup_ldweights(nc) -> int:
    """Remove InstLdweights whose weights AP matches the previous load in
    the same block (the PE array retains its stationary operand)."""
    removed = 0
    for fn in nc.m.functions:
        for blk in fn.blocks:
            insts = list(blk.instructions)
            cur_key = None
            drop = []
            for idx, inst in enumerate(insts):
                tn = type(inst).__name__
                if tn == "InstLdweights":
                    ap = inst.ins[0]
                    key = (
                        ap.memref, ap.offset, str(ap.ap), str(ap.dtype),
                        str(inst.perf_mode), str(inst.is_transpose),
                        str(inst.tile_position), str(inst.tile_size),
                    )
                    si = inst.sync_info
                    clean = si is None or (not si.on_wait and not si.on_update)
                    if key == cur_key and clean:
                        drop.append(idx)
                        removed += 1
                        continue
                    cur_key = key
                elif tn == "InstMatmult":
                    if getattr(inst, "is_transpose", None):
                        cur_key = None
            for idx in reversed(drop):
                del blk.instructions[idx]
    return removed


def build_nc(reps: int = 1, skew: int = 2, static_reps: int = 1):
    """Build the per-core program.  reps>1 wraps the body in an on-chip
    loop (For_i) for device-time measurement; skew is the pipeline depth
    in tile-pairs between selection/squares and main matmuls."""
    f32, f16 = mybir.dt.float32, mybir.dt.float16
    nc = bacc.Bacc("TRN2", target_bir_lowering=False)
    x_d = nc.dram_tensor("x_loc", [B_LOC, C, H, WIDTH], f16, kind="ExternalInput")
    a_d = nc.dram_tensor("aselT", [72, GC], f16, kind="ExternalInput")
    w_d = nc.dram_tensor("w2T", [6 * GC, O], f16, kind="ExternalInput")
    o_d = nc.dram_tensor("out_loc", [B_LOC, O, HO, WO], f32, kind="ExternalOutput")

    LFULL = HO * 64  # 3968

    with TileContext(nc) as tc:
        with (
            tc.tile_pool(name="const", bufs=1) as cpool,
            tc.tile_pool(name="xin", bufs=X_BUFS) as xpool,
            tc.tile_pool(name="packs", bufs=PK_BUFS) as packpool,
            tc.tile_pool(name="gfull", bufs=GF_BUFS) as gfpool,
            tc.tile_pool(name="gsel", bufs=GS_BUFS) as gspool,
            tc.tile_pool(name="obuf", bufs=O_BUFS) as opool,
            tc.tile_pool(name="ps_sel", bufs=2, space="PSUM") as pspool,
            tc.tile_pool(name="ps_out", bufs=2, space="PSUM") as popool,
        ):
            a_r = cpool.tile([72, GC], f16, tag="a_r")
            nc.sync.dma_start(a_r[:], a_d[:])

            def load_x(x_t, b, h, col0, col1):
                for di in range(3):
                    hi = min(col1, H * WIDTH - di * 64 - 2)
                    if hi > col0:
                        ap = _x_window_ap(x_d, b, h, di, hi - col0)
                        ap.offset += col0
                        nc.sync.dma_start(
                            x_t[di * 24 : (di + 1) * 24, col0:hi], ap
                        )
                    if hi < col1:
                        nc.sync.dma_start(
                            x_t[di * 24 : (di + 1) * 24, hi:col1],
                            _x_window_ap(x_d, b, h, 0, col1 - hi),
                        )

            xr_all = [
                [
                    xpool.tile([72, LFULL], f16, tag=f"x{b}{h}", name=f"x{b}{h}")
                    for h in range(2)
                ]
                for b in range(B_LOC)
            ]
            for h in range(2):
                load_x(xr_all[0][h], 0, h, 0, 1024)
            w_r = cpool.tile([GC, 6, O], f16, tag="w_r")
            nc.sync.dma_start(w_r[:], w_d[:].rearrange("(k p) o -> p k o", p=GC))
            for h in range(2):
                load_x(xr_all[0][h], 0, h, 1024, LFULL)
            for b in range(1, B_LOC):
                for h in range(2):
                    load_x(xr_all[b][h], b, h, 0, LFULL)

            gconst = None
            sconst = None
            if "const_g" in ABLATE or "const_selg" in ABLATE:
                if "const_g" in ABLATE:
                    gconst = [cpool.tile([GC, LFULL], f16, tag=f"gc{i}",
                                         name=f"gc{i}") for i in range(2)]
                    for t in gconst:
                        nc.vector.memset(t[:], 0.01)
                if "const_selg" in ABLATE:
                    sconst = cpool.tile([GC, 2, 512], f16, tag="sc")
                    nc.vector.memset(sconst[:], 0.01)
            if "no_mains" in ABLATE:
                o_zero = cpool.tile([O, 8 * WO], f32, tag="o_zero")
                nc.vector.memset(o_zero[:], 0.0)
                for b in range(B_LOC):
                    for ho0, nr in [(i * 8, 8) for i in range(7)] + [(56, 6)]:
                        nc.sync.dma_start(
                            o_d[b, :, ho0 : ho0 + nr, :], o_zero[:, : nr * WO]
                        )

            hoisted_packs = {}
            if "hoist_packs" in ABLATE:
                for b in range(B_LOC):
                    for h in range(2):
                        x_t = xr_all[b][h]
                        pks = (
                            cpool.tile([64, LFULL], f16, tag=f"hp1{b}{h}",
                                       name=f"hp1{b}{h}"),
                            cpool.tile([64, LFULL], f16, tag=f"hp2{b}{h}",
                                       name=f"hp2{b}{h}"),
                            cpool.tile([64, LFULL], f16, tag=f"hp3{b}{h}",
                                       name=f"hp3{b}{h}"),
                        )
                        nc.sync.dma_start(pks[0][:, :], x_t[8:72, :])
                        nc.sync.dma_start(pks[1][0:56, :], x_t[16:72, :])
                        nc.sync.dma_start(pks[1][56:64, :], x_t[0:8, :])
                        nc.sync.dma_start(pks[2][0:48, :], x_t[24:72, :])
                        nc.sync.dma_start(pks[2][48:64, :], x_t[0:16, :])
                        hoisted_packs[(b, h)] = pks

            def make_products(b, h):
                """Full-width product chunks 0/1 for (batch, half)."""
                x_t = xr_all[b][h]
                # base-0 staged copies of the shifted B operands (TensorTensor
                # requires equal base partitions for both SBUF inputs)
                # pk2/pk3 are widened to 64 rows with valid junk so the
                # product ops also fill the zero-weight pad rows of the g
                # chunks (uninitialized SBUF could hold NaN; 0*NaN = NaN)
                if "hoist_packs" in ABLATE:
                    pk1, pk2, pk3 = hoisted_packs[(b, h)]
                    g0 = gfpool.tile([GC, LFULL], f16, tag="gf", name="g0")
                    g1 = gfpool.tile([GC, LFULL], f16, tag="gf", name="g1")
                    nc.vector.tensor_mul(g0[0:64, :], x_t[0:64, :], pk1[:, :])
                    nc.vector.tensor_mul(g0[64:128, :], x_t[0:64, :], pk2[:, :])
                    nc.vector.tensor_mul(g1[0:64, :], x_t[0:64, :],
                                         x_t[0:64, :])
                    nc.vector.tensor_mul(g1[64:128, :], x_t[0:64, :],
                                         pk3[:, :])
                    return (g0, g1)
                pk1 = packpool.tile([64, LFULL], f16, tag="pk1", name="pk1")
                pk2 = packpool.tile([64, LFULL], f16, tag="pk2", name="pk2")
                pk3 = packpool.tile([64, LFULL], f16, tag="pk3", name="pk3")
                nc.sync.dma_start(pk1[:, :], x_t[8:72, :])
                nc.sync.dma_start(pk2[0:56, :], x_t[16:72, :])
                nc.sync.dma_start(pk2[56:64, :], x_t[0:8, :])
                nc.sync.dma_start(pk3[0:48, :], x_t[24:72, :])
                nc.sync.dma_start(pk3[48:64, :], x_t[0:16, :])
                g0 = gfpool.tile([GC, LFULL], f16, tag="gf", name="g0")
                g1 = gfpool.tile([GC, LFULL], f16, tag="gf", name="g1")
                nc.vector.tensor_mul(g0[0:64, :], x_t[0:64, :], pk1[:, :])
                nc.vector.tensor_mul(g0[64:128, :], x_t[0:64, :], pk2[:, :])
                if D0_SPLIT and h == 0:
                    nc.scalar.square(g1[0:64, :], x_t[0:64, :])
                else:
                    d0eng = nc.gpsimd if D0_GPS else nc.vector
                    d0eng.tensor_mul(g1[0:64, :], x_t[0:64, :], x_t[0:64, :])
                nc.vector.tensor_mul(g1[64:128, :], x_t[0:64, :], pk3[:, :])
                return (g0, g1)

            eng_busy = {"act": 0.0, "dve": 0.0}

            def out_copy(o_view, ps_view, cost_act, cost_dve):
                if eng_busy["act"] + cost_act < eng_busy["dve"] + cost_dve:
                    nc.scalar.copy(o_view, ps_view)
                    eng_busy["act"] += cost_act
                else:
                    nc.vector.tensor_copy(o_view, ps_view)
                    eng_busy["dve"] += cost_dve

            def do_mains(st):
                b, ptiles, gchunks, selg = st
                if "no_mains" in ABLATE:
                    return
                ps_o = popool.tile([O, 2, 512], f32, tag="ps_o", name="ps_o")
                for h in range(2):
                    for ch in range(3):
                        kk = h * 3 + ch
                        for j, (ho0, nr) in enumerate(ptiles):
                            lt = nr * 64
                            if ch == 2:
                                mov = selg[j][:, h, :lt]
                            else:
                                c0 = ho0 * 64
                                mov = gchunks[h][ch][:, c0 : c0 + lt]
                            nc.tensor.matmul(
                                ps_o[:, j, :lt],
                                w_r[:, kk, :],
                                mov,
                                start=(kk == 0),
                                stop=(kk == 5),
                            )
                o_t = opool.tile([O, 2, 8 * WO], f32, tag="o", name="o_t")
                nr0, nr1 = ptiles[0][1], ptiles[1][1]
                if nr0 == nr1:
                    ps_view = ps_o[:].rearrange("o j (r w) -> o j r w", w=64)
                    o_view = o_t[:, :, : nr0 * WO].rearrange(
                        "o j (r w) -> o j r w", w=WO
                    )
                    out_copy(o_view, ps_view[:, :, :nr0, :WO], 1.92, 1.59)
                else:
                    for j, (ho0, nr) in enumerate(ptiles):
                        ps_view = ps_o[:, j, : nr * 64].rearrange(
                            "o (r w) -> o r w", w=64
                        )
                        o_view = o_t[:, j, : nr * WO].rearrange(
                            "o (r w) -> o r w", w=WO
                        )
                        out_copy(o_view, ps_view[:, :, :WO], 1.1, 0.95)
                # paired store: rows ho0..ho0+nr are contiguous in o_d
                # (tile j=1's rows sit at l = 8*WO in the flattened o_t)
                ho0 = ptiles[0][0]
                nr = nr0 + nr1
                nc.sync.dma_start(
                    o_d[b, :, ho0 : ho0 + nr, :],
                    o_t[:].rearrange("o j l -> o (j l)")[:, : nr * WO],
                )

            def warmup():
                for i in range(12):
                    ps_w = popool.tile([O, 2, 512], f32, tag="ps_o", name="warm")
                    nc.tensor.matmul(
                        ps_w[:, 0, :512], a_r[:, :128], xr_all[0][0][:, :512],
                        start=True, stop=True,
                    )

            def body(it=None, unroll=1):
                pending = []
                for b in range(B_LOC):
                    if "const_g" in ABLATE:
                        gchunks = [gconst, gconst]
                    else:
                        gchunks = [make_products(b, h) for h in range(2)]
                    for ptiles in TILE_PAIRS:
                        if "const_selg" in ABLATE:
                            pending.append((b, ptiles, gchunks,
                                            [sconst, sconst]))
                            if len(pending) > skew:
                                do_mains(pending.pop(0))
                            continue
                        selg = []
                        for j, (ho0, nr) in enumerate(ptiles):
                            lt = nr * 64
                            c0 = ho0 * 64
                            ps_s = pspool.tile(
                                [GC, 2, 512], f32, tag="ps_s", name="ps_s"
                            )
                            for h in range(2):
                                nc.tensor.matmul(
                                    ps_s[:, h, :lt],
                                    a_r[:],
                                    xr_all[b][h][:, c0 : c0 + lt],
                                    start=True,
                                    stop=True,
                                )
                            g_t = gspool.tile([GC, 2, 512], f16, tag="gs",
                                              name="g_t")
                            nc.scalar.square(g_t[:, :, :lt], ps_s[:, :, :lt])
                            selg.append(g_t)
                        pending.append((b, ptiles, gchunks, selg))
                        if len(pending) > skew:
                            do_mains(pending.pop(0))
                for st in pending:
                    do_mains(st)

            warmup()
            if static_reps > 1:
                for _ in range(static_reps):
                    body()
            elif reps == 1:
                body()
            else:
                hint = (
                    mybir.EngineType.PE,
                    mybir.EngineType.Activation,
                    mybir.EngineType.DVE,
                    mybir.EngineType.SP,
                    mybir.EngineType.Pool,
                )
                with tc.For_i(0, reps, 1, hint_engines=hint) as _it:
                    body()
    if DEDUP_LDW:
        build_nc.last_dedup = _dedup_ldweights(nc)
    nc.compile()
    return nc


@functools.lru_cache(maxsize=1)
def _cached_nc():
    return build_nc()


def make_in_maps(x: np.ndarray, W: np.ndarray) -> list:
    x = np.asarray(x, dtype=np.float32)
    W = np.asarray(W, dtype=np.float32)
    AselT, W2T = _build_consts(W)
    x_r = x.astype(np.float16)
    return [
        {
            "x_loc": np.ascontiguousarray(x_r[k * B_LOC : (k + 1) * B_LOC]),
            "aselT": AselT,
            "w2T": W2T,
        }
        for k in range(N_CORES)
    ]


def kernel(x: np.ndarray, W: np.ndarray, _trace: bool = False):
    nc = _cached_nc()
    in_maps = make_in_maps(x, W)
    try:
        r = run_bass_kernel_spmd(
            nc, in_maps, core_ids=list(range(N_CORES)), trace=_trace
        )
    except Exception:
        # transient NRT_EXEC_UNIT_UNRECOVERABLE has been observed once on
        # this fabric; a fresh attempt recovers
        r = run_bass_kernel_spmd(
            nc, in_maps, core_ids=list(range(N_CORES)), trace=_trace
        )
    out = np.concatenate([m["out_loc"] for m in r.results], axis=0)
    if _trace:
        kernel.last_result = r
    return out


if __name__ == "__main__":
    rng = np.random.default_rng(0)
    x = rng.standard_normal((B, C, H, WIDTH), dtype=np.float32)
    W = rng.standard_normal((O, C * 81), dtype=np.float32)
    out = kernel(x, W)
    print("out shape", out.shape, out.dtype)
